# revision 7
# baseline (speedup 1.0000x reference)
"""Trainium2 Bass kernel for nn_Hallucigraph (GAE-style GNN), 8-core SPMD.

Sharding: node dim N row-sharded across 8 cores. All big matmuls keep the
node dim on the moving/stationary side so no on-device transposes of NxN
data are needed; host provides transposed shards (adjT, noiseT, xT).
adj_scores is symmetric, so each core computes scores[:, I_k] (a column
block) contiguously and the host concatenates along axis=1.
"""
import sys
sys.path.insert(0, "/opt/trn_rl_repo")
import numpy as np
import concourse.bass as bass
import concourse.mybir as mybir
from concourse import bacc
from concourse.tile import TileContext
from concourse.bass_utils import run_bass_kernel_spmd

F32 = mybir.dt.float32
F32R = mybir.dt.float32r
BF16 = mybir.dt.bfloat16
AF = mybir.ActivationFunctionType
OP = mybir.AluOpType
AX = mybir.AxisListType

NCORES = 8
N = 6144          # nodes
F = 512           # input features
H = 256           # hidden
C = 40            # classes
TEMP = 0.25
ALPHA = 0.5
EPS = 1e-10

S = N // NCORES          # 768 rows per core
NB = N // 128            # 48 node blocks
SB = S // 128            # 6 node blocks per core
FB = F // 128            # 4 feature blocks
HB = H // 128            # 2 hidden blocks
RG = [list(range(NCORES))]

SPLITS = [(0, 512), (512, S - 512)] if S > 512 else [(0, S)]


def ts128(i):
    return bass.ts(i, 128)


def ds128(i):
    return bass.ds(i * 128, 128)


def build():
    nc = bacc.Bacc(None, target_bir_lowering=False)

    # ---- per-core external inputs ----
    adjT = nc.dram_tensor("adjT", [N, S], F32R, kind="ExternalInput")
    adjTbf = nc.dram_tensor("adjTbf", [N, S], BF16, kind="ExternalInput")
    noiseT = nc.dram_tensor("noiseT", [N, S], BF16, kind="ExternalInput")
    xTb = nc.dram_tensor("xTb", [NB, 128, FB, 128], F32R, kind="ExternalInput")
    xTs = nc.dram_tensor("xTs", [F, S], F32R, kind="ExternalInput")
    w1 = nc.dram_tensor("w1", [F, H], F32R, kind="ExternalInput")
    w2 = nc.dram_tensor("w2", [H, H], F32R, kind="ExternalInput")
    wd1 = nc.dram_tensor("wd1", [H, H], F32R, kind="ExternalInput")
    wd2 = nc.dram_tensor("wd2", [F, H], F32R, kind="ExternalInput")
    wd3 = nc.dram_tensor("wd3", [H, H], BF16, kind="ExternalInput")
    w3 = nc.dram_tensor("w3", [H, C], BF16, kind="ExternalInput")

    scores_out = nc.dram_tensor("scores_out", [N, S], F32, kind="ExternalOutput")
    preds_out = nc.dram_tensor("preds_out", [S, C], F32, kind="ExternalOutput")

    h_in = nc.dram_tensor("h_in", [H, S], F32R)
    h_out = nc.dram_tensor("h_out", [NCORES * H, S], F32R, addr_space="Shared")
    z0_in = nc.dram_tensor("z0_in", [H, S], F32R)
    z0_out = nc.dram_tensor("z0_out", [NCORES * H, S], F32R, addr_space="Shared")
    d_in = nc.dram_tensor("d_in", [1, S], F32)
    d_out = nc.dram_tensor("d_out", [NCORES, S], F32, addr_space="Shared")
    z12_in = nc.dram_tensor("z12_in", [H, S], BF16)
    z12_out = nc.dram_tensor("z12_out", [NCORES * H, S], BF16, addr_space="Shared")
    u_in = nc.dram_tensor("u_in", [S, C], BF16)
    u_out = nc.dram_tensor("u_out", [NCORES * S, C], BF16, addr_space="Shared")
    s2_spill = nc.dram_tensor("s2_spill", [128, NB, H], BF16)

    def ag(bounce_in, bounce_out):
        nc.gpsimd.collective_compute(
            "AllGather", OP.bypass, replica_groups=RG,
            ins=[bounce_in[:, :].opt()], outs=[bounce_out[:, :].opt()])

    with TileContext(nc) as tc:
        keep_cm = tc.tile_pool(name="keep", bufs=1)
        keep = keep_cm.__enter__()
        # ---- weights ----
        w1_t = keep.tile([128, FB, H], F32R, tag="w1")
        nc.sync.dma_start(out=w1_t[:, :, :], in_=w1.rearrange("(b p) h -> p b h", p=128))
        w2_t = keep.tile([128, HB, H], F32R, tag="w2")
        nc.sync.dma_start(out=w2_t[:, :, :], in_=w2.rearrange("(b p) h -> p b h", p=128))
        wd1_t = keep.tile([128, HB, H], F32R, tag="wd1")
        nc.sync.dma_start(out=wd1_t[:, :, :], in_=wd1.rearrange("(b p) h -> p b h", p=128))
        wd2_t = keep.tile([128, FB, H], F32R, tag="wd2")
        nc.sync.dma_start(out=wd2_t[:, :, :], in_=wd2.rearrange("(b p) h -> p b h", p=128))
        wd3_t = keep.tile([128, HB, H], BF16, tag="wd3")
        nc.sync.dma_start(out=wd3_t[:, :, :], in_=wd3.rearrange("(b p) h -> p b h", p=128))
        w3_t = keep.tile([128, HB, C], BF16, tag="w3")
        nc.sync.dma_start(out=w3_t[:, :, :], in_=w3.rearrange("(b p) c -> p b c", p=128))
        ones_bf = keep.tile([128, 1], BF16, tag="ones")
        nc.vector.memset(ones_bf[:, :], 1.0)
        ones1 = keep.tile([1, 128], F32, tag="ones1")
        nc.vector.memset(ones1[:, :], 1.0)

        s2td_pre = keep.tile([128, HB, S], F32, tag="s2tdp")
        z0s_t = keep.tile([128, HB, S], F32R, tag="z0s")
        dcol = keep.tile([128, NB], F32, tag="dcol")
        dbc = keep.tile([128, S], F32, tag="dbc")
        z12s_t = keep.tile([128, HB, S], BF16, tag="z12s")

        # ============ Phase A ============
        sa_cm = tc.tile_pool(name="sa", bufs=1)
        sap = sa_cm.__enter__()
        s_a = sap.tile([128, NB, H], F32R, tag="sa")
        s2_cm = tc.tile_pool(name="s2p", bufs=1)
        s2p = s2_cm.__enter__()
        s2pre = s2p.tile([128, NB, H], BF16, tag="s2pre")
        xts_cm = tc.tile_pool(name="xtsp", bufs=1)
        xtsp = xts_cm.__enter__()
        xTs_t = xtsp.tile([128, FB, S], F32R, tag="xTs")
        nc.sync.dma_start(out=xTs_t[:, :, :], in_=xTs.rearrange("(b p) s -> p b s", p=128))

        with tc.tile_pool(name="xin", bufs=3) as xin, \
             tc.tile_pool(name="psA", bufs=2, space="PSUM") as psA:
            for mb in range(NB):
                xt = xin.tile([128, FB, 128], F32R, tag="xt")
                nc.sync.dma_start(out=xt[:, :, :], in_=xTb[mb, :, :, :])
                p1 = psA.tile([128, H], F32, tag="p1")
                p2 = psA.tile([128, H], F32, tag="p2")
                for fb in range(FB):
                    nc.tensor.matmul(p1[:, :], xt[:, fb, :], w1_t[:, fb, :],
                                     start=(fb == 0), stop=(fb == FB - 1))
                for fb in range(FB):
                    nc.tensor.matmul(p2[:, :], xt[:, fb, :], wd2_t[:, fb, :],
                                     start=(fb == 0), stop=(fb == FB - 1))
                nc.scalar.copy(s_a[:, mb, :], p1[:, :])
                nc.scalar.copy(s2pre[:, mb, :], p2[:, :])
            for ch in range(HB):
                pd = psA.tile([128, S], F32, tag="pdA")
                for fb in range(FB):
                    for off, w in SPLITS:
                        nc.tensor.matmul(pd[:, off:off + w],
                                         wd2_t[:, fb, ts128(ch)], xTs_t[:, fb, off:off + w],
                                         start=(fb == 0), stop=(fb == FB - 1))
                nc.scalar.copy(s2td_pre[:, ch, :], pd[:, :])
        xts_cm.__exit__(None, None, None)
        nc.sync.dma_start(out=s2_spill[:, :, :], in_=s2pre[:, :, :])
        s2_cm.__exit__(None, None, None)

        # ============ Phase B ============
        hs_cm = tc.tile_pool(name="hs", bufs=1)
        hsp = hs_cm.__enter__()
        h_sT = hsp.tile([128, HB, S], F32R, tag="hsT")
        with tc.tile_pool(name="adj1", bufs=3) as adj1, \
             tc.tile_pool(name="psB", bufs=1, space="PSUM") as psB:
            ph = [psB.tile([128, S], F32, tag=f"ph{ch}", name=f"ph{ch}") for ch in range(HB)]
            for mb in range(NB):
                at = adj1.tile([128, S], F32R, tag="at")
                nc.sync.dma_start(out=at[:, :], in_=adjT[ds128(mb), :])
                for ch in range(HB):
                    for off, w in SPLITS:
                        nc.tensor.matmul(ph[ch][:, off:off + w],
                                         s_a[:, mb, ts128(ch)], at[:, off:off + w],
                                         start=(mb == 0), stop=(mb == NB - 1))
            for ch in range(HB):
                nc.scalar.activation(h_sT[:, ch, :], ph[ch][:, :], AF.Relu)

        nc.sync.dma_start(out=h_in.rearrange("(b p) s -> p b s", p=128), in_=h_sT[:, :, :])
        ag(h_in, h_out)
        hs_cm.__exit__(None, None, None)
        sa_cm.__exit__(None, None, None)

        sb_cm = tc.tile_pool(name="sbp", bufs=1)
        sbp = sb_cm.__enter__()
        s_b = sbp.tile([128, NB, H], F32R, tag="sb")
        hT_cm = tc.tile_pool(name="hT", bufs=1)
        hTp = hT_cm.__enter__()
        hT = hTp.tile([128, HB, N], F32R, tag="hT")
        for r in range(NCORES):
            for ch in range(HB):
                nc.sync.dma_start(
                    out=hT[:, ch, r * S:(r + 1) * S],
                    in_=h_out[r * H + ch * 128: r * H + (ch + 1) * 128, :])
        with tc.tile_pool(name="psB2", bufs=2, space="PSUM") as psB2:
            for mb in range(NB):
                pb = psB2.tile([128, H], F32, tag="pb")
                for ch in range(HB):
                    nc.tensor.matmul(pb[:, :], hT[:, ch, ds128(mb)], w2_t[:, ch, :],
                                     start=(ch == 0), stop=(ch == HB - 1))
                nc.scalar.copy(s_b[:, mb, :], pb[:, :])
        hT_cm.__exit__(None, None, None)

        with tc.tile_pool(name="adj2", bufs=3) as adj2, \
             tc.tile_pool(name="psB3", bufs=1, space="PSUM") as psB3:
            pz = [psB3.tile([128, S], F32, tag=f"pz{ch}", name=f"pz{ch}") for ch in range(HB)]
            for mb in range(NB):
                at = adj2.tile([128, S], F32R, tag="at2")
                nc.sync.dma_start(out=at[:, :], in_=adjT[ds128(mb), :])
                for ch in range(HB):
                    for off, w in SPLITS:
                        nc.tensor.matmul(pz[ch][:, off:off + w],
                                         s_b[:, mb, ts128(ch)], at[:, off:off + w],
                                         start=(mb == 0), stop=(mb == NB - 1))
            for ch in range(HB):
                nc.scalar.copy(z0s_t[:, ch, :], pz[ch][:, :])
        sb_cm.__exit__(None, None, None)

        nc.sync.dma_start(out=z0_in.rearrange("(b p) s -> p b s", p=128), in_=z0s_t[:, :, :])
        ag(z0_in, z0_out)

        # ============ Phase C: scoresT, aT, d ============
        aT_cm = tc.tile_pool(name="aTp", bufs=1)
        aTp = aT_cm.__enter__()
        aT = aTp.tile([128, NB, S], BF16, tag="aT")
        with tc.tile_pool(name="zl", bufs=4) as zl, \
             tc.tile_pool(name="nin", bufs=3) as nin, \
             tc.tile_pool(name="sco", bufs=2) as sco, \
             tc.tile_pool(name="psC", bufs=2, space="PSUM") as psC, \
             tc.tile_pool(name="psR", bufs=1, space="PSUM") as psR:
            prs = psR.tile([1, S], F32, tag="prs")
            for jb in range(NB):
                r, xx = jb // SB, jb % SB
                zt = zl.tile([128, HB, 128], F32R, tag="zt")
                for ch in range(HB):
                    nc.sync.dma_start(
                        out=zt[:, ch, :],
                        in_=z0_out[r * H + ch * 128: r * H + (ch + 1) * 128,
                                   xx * 128:(xx + 1) * 128])
                ps = psC.tile([128, S], F32, tag="ps")
                for ch in range(HB):
                    for off, w in SPLITS:
                        nc.tensor.matmul(ps[:, off:off + w],
                                         zt[:, ch, :], z0s_t[:, ch, off:off + w],
                                         start=(ch == 0), stop=(ch == HB - 1))
                so = sco.tile([128, S], F32, tag="so")
                nc.scalar.copy(so[:, :], ps[:, :])
                nc.sync.dma_start(out=scores_out[ds128(jb), :], in_=so[:, :])
                nt = nin.tile([128, S], BF16, tag="nt")
                nc.sync.dma_start(out=nt[:, :], in_=noiseT[ds128(jb), :])
                st = sco.tile([128, S], F32, tag="st")
                nc.vector.scalar_tensor_tensor(st[:, :], ps[:, :], 1.0 / TEMP,
                                               nt[:, :], OP.mult, OP.add)
                nc.scalar.activation(aT[:, jb, :], st[:, :], AF.Sigmoid)
                for off, w in SPLITS:
                    nc.tensor.matmul(prs[:, off:off + w], ones_bf[:, :],
                                     aT[:, jb, off:off + w],
                                     start=(jb == 0), stop=(jb == NB - 1))
            ds_t = sco.tile([1, S], F32, tag="dst")
            nc.vector.tensor_scalar_add(ds_t[:, :], prs[:, :], 1.0)
            nc.scalar.sqrt(ds_t[:, :], ds_t[:, :])
            nc.vector.reciprocal(ds_t[:, :], ds_t[:, :])
            nc.sync.dma_start(out=d_in[:, :], in_=ds_t[:, :])
            ag(d_in, d_out)
            pbc = psR.tile([128, S], F32, tag="pbc")
            for off, w in SPLITS:
                nc.tensor.matmul(pbc[:, off:off + w], ones1[:, :], ds_t[:, off:off + w],
                                 start=True, stop=True)
            nc.vector.tensor_copy(dbc[:, :], pbc[:, :])
        nc.sync.dma_start(out=dcol[:, :],
                          in_=d_out.rearrange("r (x p) -> p (r x)", p=128))

        # ============ Phase D ============
        s12_cm = tc.tile_pool(name="s12", bufs=1)
        s12p = s12_cm.__enter__()
        s12d = s12p.tile([128, NB, 2 * H], BF16, tag="s12d")
        s1td = keep.tile([128, HB, S], F32, tag="s1td")
        with tc.tile_pool(name="zl2", bufs=4) as zl2, \
             tc.tile_pool(name="psD", bufs=2, space="PSUM") as psD:
            for mb in range(NB):
                zt = zl2.tile([128, HB, 128], F32R, tag="zt2")
                r, xx = mb // SB, mb % SB
                for ch in range(HB):
                    nc.sync.dma_start(
                        out=zt[:, ch, :],
                        in_=z0_out[r * H + ch * 128: r * H + (ch + 1) * 128,
                                   xx * 128:(xx + 1) * 128])
                p1 = psD.tile([128, H], F32, tag="pd1")
                for ch in range(HB):
                    nc.tensor.matmul(p1[:, :], zt[:, ch, :], wd1_t[:, ch, :],
                                     start=(ch == 0), stop=(ch == HB - 1))
                nc.scalar.activation(s12d[:, mb, 0:H], p1[:, :], AF.Copy,
                                     scale=dcol[:, mb:mb + 1])
                s2t = zl2.tile([128, H], BF16, tag="s2t")
                nc.sync.dma_start(out=s2t[:, :], in_=s2_spill[:, mb, :])
                nc.scalar.activation(s12d[:, mb, H:2 * H], s2t[:, :], AF.Copy,
                                     scale=dcol[:, mb:mb + 1])
            for ch in range(HB):
                pd = psD.tile([128, S], F32, tag="pdd")
                for c2 in range(HB):
                    for off, w in SPLITS:
                        nc.tensor.matmul(pd[:, off:off + w],
                                         wd1_t[:, c2, ts128(ch)], z0s_t[:, c2, off:off + w],
                                         start=(c2 == 0), stop=(c2 == HB - 1))
                nc.vector.tensor_mul(s1td[:, ch, :], pd[:, :], dbc[:, :])
                nc.vector.tensor_mul(s2td_pre[:, ch, :], s2td_pre[:, ch, :], dbc[:, :])

        with tc.tile_pool(name="psY", bufs=1, space="PSUM") as psY, \
             tc.tile_pool(name="ev", bufs=1) as ev:
            py = [psY.tile([128, S], F32, tag=f"py{cc}", name=f"py{cc}") for cc in range(4)]
            for jb in range(NB):
                for cc in range(4):
                    for off, w in SPLITS:
                        nc.tensor.matmul(py[cc][:, off:off + w],
                                         s12d[:, jb, ts128(cc)], aT[:, jb, off:off + w],
                                         start=(jb == 0), stop=(jb == NB - 1))
            for ch in range(HB):
                t1 = ev.tile([128, S], F32, tag="t1")
                nc.vector.tensor_add(t1[:, :], py[ch][:, :], s1td[:, ch, :])
                nc.vector.tensor_mul(t1[:, :], t1[:, :], dbc[:, :])
                z1 = ev.tile([128, S], F32, tag="z1")
                nc.scalar.activation(z1[:, :], t1[:, :], AF.Relu)
                t2 = ev.tile([128, S], F32, tag="t2")
                nc.vector.tensor_add(t2[:, :], py[2 + ch][:, :], s2td_pre[:, ch, :])
                nc.vector.tensor_mul(t2[:, :], t2[:, :], dbc[:, :])
                z2 = ev.tile([128, S], F32, tag="z2")
                nc.scalar.activation(z2[:, :], t2[:, :], AF.Relu)
                nc.vector.tensor_add(z12s_t[:, ch, :], z1[:, :], z2[:, :])
        s12_cm.__exit__(None, None, None)

        nc.sync.dma_start(out=z12_in.rearrange("(b p) s -> p b s", p=128), in_=z12s_t[:, :, :])
        ag(z12_in, z12_out)

        # ============ Phase E ============
        s3_cm = tc.tile_pool(name="s3p", bufs=1)
        s3p = s3_cm.__enter__()
        s3d = s3p.tile([128, NB, H], BF16, tag="s3d")
        z12_cm = tc.tile_pool(name="z12p", bufs=1)
        z12pp = z12_cm.__enter__()
        z12T = z12pp.tile([128, HB, N], BF16, tag="z12T")
        for r in range(NCORES):
            for ch in range(HB):
                nc.sync.dma_start(
                    out=z12T[:, ch, r * S:(r + 1) * S],
                    in_=z12_out[r * H + ch * 128: r * H + (ch + 1) * 128, :])
        s3td = keep.tile([128, HB, S], F32, tag="s1td")  # reuse slot of s1td
        with tc.tile_pool(name="psE", bufs=2, space="PSUM") as psE:
            for mb in range(NB):
                p1 = psE.tile([128, H], F32, tag="pe1")
                for ch in range(HB):
                    nc.tensor.matmul(p1[:, :], z12T[:, ch, ds128(mb)], wd3_t[:, ch, :],
                                     start=(ch == 0), stop=(ch == HB - 1))
                nc.scalar.activation(s3d[:, mb, :], p1[:, :], AF.Copy,
                                     scale=dcol[:, mb:mb + 1])
            for ch in range(HB):
                pd = psE.tile([128, S], F32, tag="ped")
                for c2 in range(HB):
                    for off, w in SPLITS:
                        nc.tensor.matmul(pd[:, off:off + w],
                                         wd3_t[:, c2, ts128(ch)], z12s_t[:, c2, off:off + w],
                                         start=(c2 == 0), stop=(c2 == HB - 1))
                nc.vector.tensor_mul(s3td[:, ch, :], pd[:, :], dbc[:, :])
        z12_cm.__exit__(None, None, None)

        zT = keep.tile([128, HB, S], BF16, tag="zT")
        with tc.tile_pool(name="psY3", bufs=1, space="PSUM") as psY3, \
             tc.tile_pool(name="ev3", bufs=1) as ev3:
            py3 = [psY3.tile([128, S], F32, tag=f"py3{ch}", name=f"py3{ch}") for ch in range(HB)]
            for jb in range(NB):
                for ch in range(HB):
                    for off, w in SPLITS:
                        nc.tensor.matmul(py3[ch][:, off:off + w],
                                         s3d[:, jb, ts128(ch)], aT[:, jb, off:off + w],
                                         start=(jb == 0), stop=(jb == NB - 1))
            for ch in range(HB):
                t1 = ev3.tile([128, S], F32, tag="t31")
                nc.vector.tensor_add(t1[:, :], py3[ch][:, :], s3td[:, ch, :])
                nc.vector.tensor_mul(t1[:, :], t1[:, :], dbc[:, :])
                # z = alpha*z3 + (1-alpha)*z0
                t2 = ev3.tile([128, S], F32, tag="t32")
                nc.vector.tensor_add(t2[:, :], t1[:, :], z0s_t[:, ch, :])
                nc.vector.tensor_scalar_mul(zT[:, ch, :], t2[:, :], ALPHA)
        s3_cm.__exit__(None, None, None)
        aT_cm.__exit__(None, None, None)

        us_t = keep.tile([128, SB, C], BF16, tag="us")
        with tc.tile_pool(name="psU", bufs=2, space="PSUM") as psU:
            for ib in range(SB):
                pu = psU.tile([128, C], F32, tag="pu")
                for ch in range(HB):
                    nc.tensor.matmul(pu[:, :], zT[:, ch, ts128(ib)], w3_t[:, ch, :],
                                     start=(ch == 0), stop=(ch == HB - 1))
                nc.vector.tensor_copy(us_t[:, ib, :], pu[:, :])
        nc.sync.dma_start(out=u_in.rearrange("(b p) c -> p b c", p=128), in_=us_t[:, :, :])
        ag(u_in, u_out)
        u_t = keep.tile([128, NB, C], BF16, tag="u")
        for r in range(NCORES):
            nc.sync.dma_start(
                out=u_t[:, r * SB:(r + 1) * SB, :],
                in_=u_out[r * S:(r + 1) * S, :].rearrange("(b p) c -> p b c", p=128))

        with tc.tile_pool(name="adj3", bufs=3) as adj3, \
             tc.tile_pool(name="psP", bufs=1, space="PSUM") as psP, \
             tc.tile_pool(name="lsm", bufs=2) as lsm:
            pp = [psP.tile([128, C], F32, tag=f"pp{ib}", name=f"pp{ib}") for ib in range(SB)]
            for kb in range(NB):
                ab = adj3.tile([128, S], BF16, tag="ab")
                nc.sync.dma_start(out=ab[:, :], in_=adjTbf[ds128(kb), :])
                for ib in range(SB):
                    nc.tensor.matmul(pp[ib][:, :], ab[:, ts128(ib)], u_t[:, kb, :],
                                     start=(kb == 0), stop=(kb == NB - 1))
            for ib in range(SB):
                mx = lsm.tile([128, 1], F32, tag="mx")
                nc.vector.reduce_max(out=mx[:, :], in_=pp[ib][:, :], axis=AX.X)
                nmx = lsm.tile([128, 1], F32, tag="nmx")
                nc.vector.tensor_scalar_mul(nmx[:, :], mx[:, :], -1.0)
                ex = lsm.tile([128, C], F32, tag="ex")
                sm = lsm.tile([128, 1], F32, tag="sm")
                nc.scalar.activation(ex[:, :], pp[ib][:, :], AF.Exp,
                                     bias=nmx[:, :], accum_out=sm[:, :])
                lsum = lsm.tile([128, 1], F32, tag="ls")
                nc.scalar.activation(lsum[:, :], sm[:, :], AF.Ln)
                tot = lsm.tile([128, 1], F32, tag="tot")
                nc.vector.tensor_add(tot[:, :], mx[:, :], lsum[:, :])
                po = lsm.tile([128, C], F32, tag="po")
                nc.vector.tensor_scalar_sub(po[:, :], pp[ib][:, :], tot[:, :])
                nc.sync.dma_start(out=preds_out[ds128(ib), :], in_=po[:, :])
        keep_cm.__exit__(None, None, None)
    nc.compile()
    return nc


_NC_CACHE = None


def _get_nc():
    global _NC_CACHE
    if _NC_CACHE is None:
        _NC_CACHE = build()
    return _NC_CACHE


def _make_noise():
    import jax
    import jax.numpy as jnp
    with jax.default_device(jax.devices("cpu")[0]):
        u = jax.random.uniform(jax.random.key(42), (N, N), dtype=jnp.float32,
                               minval=EPS, maxval=1.0 - EPS)
        noise = (jnp.log(u) - jnp.log1p(-u)) / TEMP
        return np.asarray(noise)


def kernel(x, adj, W1, W2, W3, Wd1, Wd2, Wd3):
    import ml_dtypes
    x = np.asarray(x, np.float32)
    adj = np.asarray(adj, np.float32)
    noise = _make_noise()

    xT = np.ascontiguousarray(x.T)
    xTb = np.ascontiguousarray(
        xT.reshape(FB, 128, NB, 128).transpose(2, 1, 0, 3))

    in_maps = []
    for k in range(NCORES):
        rows = slice(k * S, (k + 1) * S)
        adjT_k = np.ascontiguousarray(adj[rows, :].T)
        in_maps.append({
            "adjT": adjT_k,
            "adjTbf": adjT_k.astype(ml_dtypes.bfloat16),
            "noiseT": np.ascontiguousarray(noise[rows, :].T).astype(ml_dtypes.bfloat16),
            "xTb": xTb,
            "xTs": np.ascontiguousarray(xT[:, rows]),
            "w1": np.asarray(W1, np.float32),
            "w2": np.asarray(W2, np.float32),
            "wd1": np.asarray(Wd1, np.float32),
            "wd2": np.asarray(Wd2, np.float32),
            "wd3": np.asarray(Wd3, np.float32).astype(ml_dtypes.bfloat16),
            "w3": np.asarray(W3, np.float32).astype(ml_dtypes.bfloat16),
        })

    nc = _get_nc()
    res = run_bass_kernel_spmd(nc, in_maps, core_ids=list(range(NCORES)))
    scores = np.concatenate([r["scores_out"] for r in res.results], axis=1)
    preds = np.concatenate([r["preds_out"] for r in res.results], axis=0)
    return scores, preds
